# revision 27
# baseline (speedup 1.0000x reference)
"""GCN discriminator kernel for Trainium2 (8 NeuronCores, SPMD).

Math (matching the reference):
  deg[n]  = sum_{e: dst=n} w_e + 1
  dinv    = 1/sqrt(deg)
  norm_e  = dinv[src]*w_e*dinv[dst];  self-loop n: dinv[n]^2
  agg     = sum over incoming edges of norm_e * x[src]         [N, 128]
  h       = leaky_relu(agg @ W1 + b1)                          [N, 256]
  pooled  = segment_mean(h, batch)                             [64, 256]
  z       = leaky_relu(concat(pooled, emb[cls]) @ W2 + b2)
  out     = z @ W3 + b3                                        [64, 1]

Sharding: graph-aligned.  batch is sorted, so graphs occupy contiguous node
ranges; each core owns 8 whole graphs (LPT-balanced).  Pooling is core-local;
each core runs the tiny MLP on its own 8 graphs and the host concatenates.

Aggregation strategy: the host expands x rows into edge-slot order so the
device streams them with plain dense HWDGE DMAs — no SWDGE descriptor
emission (the original gather path was GPSIMD-emission-bound).  Self-loops
are folded in as ordinary edges (src=dst, w=dinv^2).  Edge slots are
bucketed by (dst supertile, W_SUB-wide dst subrange) so the one-hot S
matrices are [128 slots, W_SUB dsts]; x-rows and S interleave per chunk in
one contiguous stream (single DMA per granule, continuous SDMA flow).  Each
(supertile, subrange) accumulates in a PSUM column slice; per-element
has_written semantics make interleaved column-range groups safe.  Both
x-rows and S stream as fp8e4 (numerics verified offline: rel err ~1.9e-3 vs
1.77e-3 for bf16 — the bf16 downstream dominates).  The per-supertile
epilogue (PSUM evac, W1, bias, leaky, pool) is software-pipelined one
supertile behind the S-matmuls so its latency never stalls the PE.
"""

import numpy as np
import ml_dtypes

# ----------------------------------------------------------------- config
CFG = dict(
    N=50000, F=128, HID=256, G=64, NCLS=10,
    NCORES=8,
    ST_D=128,             # dsts per supertile (= PSUM agg width)
    W_SUB=32,             # dst subrange width (= S matmul free dim)
    K=128,                # edge slots per chunk (matmul contraction)
    GRAN_ST=12,           # supertiles per DMA granule (steady state)
    BUFS=3,               # stream tile-pool depth
    NEG=0.2,
    GDT="f8e4",           # x-row + S dtype: "f32" | "bf16" | "f8e4"
    MMDT="bf16",          # downstream matmul dtype
    PRELU=1,              # leaky relu via ACT Prelu (0: Relu + DVE blend)
)


def _np_dt(s):
    return {"f32": np.float32, "bf16": ml_dtypes.bfloat16,
            "f8e4": ml_dtypes.float8_e4m3}[s]


# ================================================================= host prep
class Prep:
    pass


def host_prep(inputs, cfg):
    """Integer/layout preprocessing + normalization weights.

    Returns per-core in_maps plus the static (core-independent) program
    structure.
    """
    N, F, G = cfg["N"], cfg["F"], cfg["G"]
    NC = cfg["NCORES"]
    ST_D, K, W_SUB = cfg["ST_D"], cfg["K"], cfg["W_SUB"]
    NSUB = ST_D // W_SUB
    CW = F + W_SUB                                 # stream cols per chunk
    GL = G // NC                                   # graphs per core
    GP = 32                                        # padded graph slots

    x = np.asarray(inputs["x"], np.float32)
    ei = np.asarray(inputs["edge_index"], np.int64)
    ew = np.asarray(inputs["edge_weight"], np.float32)
    batch = np.asarray(inputs["batch"], np.int64)
    cls = np.asarray(inputs["class_labels"], np.int64)
    W1 = np.asarray(inputs["W1"], np.float32)
    b1 = np.asarray(inputs["b1"], np.float32)
    emb = np.asarray(inputs["emb"], np.float32)
    W2 = np.asarray(inputs["W2"], np.float32)
    b2 = np.asarray(inputs["b2"], np.float32)
    W3 = np.asarray(inputs["W3"], np.float32)
    b3 = np.asarray(inputs["b3"], np.float32)

    HID = W1.shape[1]
    EH = emb.shape[1]

    # --- normalization weights (scalar preprocessing, O(E)) -------------
    row = ei[0]
    col = ei[1]
    deg = np.zeros(N, np.float64)
    np.add.at(deg, col, ew.astype(np.float64))
    deg += 1.0
    dinv = 1.0 / np.sqrt(deg)
    # augmented edge list: real edges + self loops
    a_src = np.concatenate([row, np.arange(N)])
    a_dst = np.concatenate([col, np.arange(N)])
    a_w = np.concatenate([
        (dinv[row] * ew.astype(np.float64) * dinv[col]).astype(np.float32),
        (dinv * dinv).astype(np.float32)])

    # --- balanced graph->core assignment (LPT bin packing, 8 per core) ----
    gcnt = np.bincount(batch, minlength=G)           # nodes per graph
    gcum = np.concatenate([[0], np.cumsum(gcnt)])    # graph start offsets
    load = np.zeros(NC, np.int64)
    nfill = np.zeros(NC, np.int64)
    asg = [[] for _ in range(NC)]                    # graphs per core
    for g in np.argsort(-gcnt, kind="stable"):
        c = min((c for c in range(NC) if nfill[c] < GL),
                key=lambda c: load[c])
        asg[c].append(int(g))
        load[c] += gcnt[g]
        nfill[c] += 1
    # in-degree incl. self loop, for slot-packing order
    indeg = np.bincount(col, minlength=N) + 1
    Dmax = int(load.max())
    NST = -(-Dmax // ST_D)      # supertiles per core
    NBK = NST * NSUB            # W_SUB-dst buckets per core

    def pack_core(d, caps):
        """Assign dsts (slot-counts d) to 32-dst buckets under caps.

        Greedy largest-that-fits per bucket; the same static cap profile
        across cores keeps the max-over-cores chunk counts tight."""
        order = np.argsort(-d, kind="stable")
        remaining = list(d[order])
        ridx = list(order)
        assign = np.empty(len(d), np.int64)
        achieved = np.zeros(len(caps), np.int64)
        for b, cap in enumerate(caps):
            if not remaining:
                break
            cnt, ssum = 0, 0
            i = 0
            while cnt < W_SUB and i < len(remaining):
                need_rest = W_SUB - cnt - 1
                tail = sum(remaining[-need_rest:]) if need_rest > 0 else 0
                if ssum + remaining[i] + tail <= cap:
                    ssum += remaining.pop(i)
                    assign[ridx.pop(i)] = b
                    cnt += 1
                else:
                    i += 1
            while cnt < W_SUB and remaining:
                ssum += remaining.pop()
                assign[ridx.pop()] = b
                cnt += 1
            achieved[b] = ssum
        return assign, achieved

    core_nls = [np.concatenate(
        [np.arange(gcum[g], gcum[g + 1]) for g in asg[c]]) for c in range(NC)]
    # static capacity profile: n5 five-chunk buckets then four-chunk ones;
    # small grid search, chunk totals decide
    base = max(0, -(-int(max(indeg[nl].sum() for nl in core_nls)
                         - NBK * 4 * K) // K))
    best = None
    for n5 in {base, base + 8, base + 16, base + 24}:
        n5 = min(n5, NBK)
        caps = np.array([5 * K] * n5 + [4 * K] * (NBK - n5))
        assigns = []
        CHp = np.zeros((NC, NBK), np.int64)
        for c in range(NC):
            a, ach = pack_core(indeg[core_nls[c]], caps)
            assigns.append(a)
            CHp[c] = -(-ach // K)
        tot = int(np.maximum(CHp.max(axis=0), 1).sum())
        if best is None or tot < best[0]:
            best = (tot, assigns, caps)
    # place small buckets first so leading granules (and the matmul
    # ramp-up) cover less data
    _, assigns, caps = best
    rank = np.empty(NBK, np.int64)
    rank[np.argsort(caps, kind="stable")] = np.arange(NBK)

    # local slot maps with holes: dst i sits at exactly
    # bucket*W_SUB + offset, partially-filled buckets leave ghost slots
    locmaps = []                                     # per-core loc -> node
    core_of_node = np.empty(N, np.int64)
    loc_of_node = np.empty(N, np.int64)
    for c in range(NC):
        nl = core_nls[c]
        fb = rank[assigns[c]]
        order = np.argsort(fb, kind="stable")
        fbo = fb[order]
        off = np.arange(len(fbo)) - np.searchsorted(fbo, fbo)
        locs = fbo * W_SUB + off
        locmap = np.full(NST * ST_D, -1, np.int64)
        locmap[locs] = nl[order]
        locmaps.append(locmap)
        core_of_node[nl[order]] = c
        loc_of_node[nl[order]] = locs

    # --- bucket edges into (core, st, sub) -------------------------------
    core_of = core_of_node[a_dst]
    dst_loc = loc_of_node[a_dst]
    st_of = dst_loc // ST_D
    sub_of = (dst_loc % ST_D) // W_SUB
    jj_of = dst_loc % W_SUB

    nbuckets = NC * NST * NSUB
    key = (core_of * NST + st_of) * NSUB + sub_of
    cnt = np.bincount(key, minlength=nbuckets).reshape(NC, NST, NSUB)
    starts = np.zeros(nbuckets + 1, np.int64)
    np.cumsum(cnt.reshape(-1), out=starts[1:])
    order = np.argsort(key, kind="stable")           # edges bucket-sorted
    e_src = a_src[order]
    e_jj = jj_of[order]
    e_w = a_w[order]

    # static chunk counts per (st, sub): max over cores, >= 1
    CH = np.maximum(-(-cnt // K), 1).max(axis=0)     # [NST, NSUB]
    TOT = int(CH.sum())                              # chunks per core

    # chunk index map in static issue order
    ci_of = np.zeros((NST, NSUB), np.int64)
    ci = 0
    for st in range(NST):
        for r in range(NSUB):
            ci_of[st, r] = ci
            ci += int(CH[st, r])

    # granules: small leading granules so the first matmuls start early,
    # then GRAN_ST supertiles per DMA for packet efficiency
    sizes = []
    st = 0
    for want in [1, 1, 1, 2, 2]:
        if st < NST:
            n = min(want, NST - st)
            sizes.append(n)
            st += n
    while st < NST:
        n = min(cfg["GRAN_ST"], NST - st)
        sizes.append(n)
        st += n
    grans = []
    st = 0
    for n in sizes:
        ci0 = int(ci_of[st, 0])
        nch = int(CH[st:st + n].sum())
        grans.append(dict(st0=st, nst=n, ci0=ci0, nch=nch))
        st += n

    static = dict(cfg=cfg, NST=NST, NSUB=NSUB, CH=CH, TOT=TOT,
                  grans=grans, ci_of=ci_of,
                  HID=HID, EH=EH, GL=GL, GP=GP, asg=asg)

    # --- per-core tensors ------------------------------------------------
    gdt = _np_dt(cfg["GDT"])
    mmdt = _np_dt(cfg["MMDT"])

    counts = np.maximum(gcnt, 1).astype(np.float64)

    # W2 row blocks: [128, 3*256] with block k = W2[k*128:(k+1)*128, :]
    w2r = np.ascontiguousarray(
        W2.reshape(3, 128, HID).transpose(1, 0, 2).reshape(128, 3 * HID))

    x_q = np.ascontiguousarray(x).astype(gdt)

    in_maps = []
    for c in range(NC):
        # slot tables [TOT, K]: src index and (jj, w) per slot
        slots_src = np.zeros((TOT, K), np.int64)
        slots_jj = np.zeros((TOT, K), np.int64)
        slots_w = np.zeros((TOT, K), np.float32)
        for st in range(NST):
            for r in range(NSUB):
                b = (c * NST + st) * NSUB + r
                s0, s1 = starts[b], starts[b + 1]
                nck = int(CH[st, r])
                ci0 = int(ci_of[st, r])
                n = s1 - s0
                assert n <= nck * K
                flat = np.arange(n)
                cix = ci0 + flat // K
                pix = flat % K
                slots_src[cix, pix] = e_src[s0:s1]
                slots_jj[cix, pix] = e_jj[s0:s1]
                slots_w[cix, pix] = e_w[s0:s1]
        # interleaved stream [128, TOT*CW]: per chunk F x-row cols + W_SUB
        # one-hot S cols
        xs = np.zeros((TOT, K, CW), np.float32)
        xs[:, :, :F] = x_q[slots_src].astype(np.float32)
        tix = np.repeat(np.arange(TOT), K)
        pix = np.tile(np.arange(K), TOT)
        xs[tix, pix, F + slots_jj.reshape(-1)] = slots_w.reshape(-1)
        xs = np.ascontiguousarray(
            xs.transpose(1, 0, 2).reshape(K, TOT * CW)).astype(gdt)

        locmap = locmaps[c]
        gslot = {g: j for j, g in enumerate(asg[c])}

        # pooling matrix [128, NST*GP], scaled by 1/count (mean pooling);
        # ghost slots (locmap < 0) stay zero
        pmat = np.zeros((128, NST * GP), np.float64)
        for loc in np.nonzero(locmap >= 0)[0]:
            g = batch[locmap[loc]]
            pmat[loc % ST_D, (loc // ST_D) * GP + gslot[g]] = 1.0 / counts[g]

        # class embeddings for this core's graphs: clt [NCLS, GP]
        clt = np.zeros((cfg["NCLS"], GP), mmdt)
        clt[cls[asg[c]], np.arange(GL)] = 1.0

        m = dict(
            xs=xs,
            pmat=pmat.astype(mmdt),
            w1=W1.astype(mmdt),
            w2r=w2r.astype(mmdt),
            b1bc=np.tile(b1.reshape(1, HID), (128, 1)).astype(mmdt),
            b2bc=np.tile(b2.reshape(1, HID), (GP, 1)).astype(mmdt),
            w3bc=np.tile(W3.reshape(1, HID), (GP, 1)).astype(mmdt),
            b3bc=np.full((GP, 1), b3[0], np.float32),
            embh=emb.astype(mmdt),
            clt=clt,
        )
        in_maps.append(m)

    prep = Prep()
    prep.static = static
    prep.in_maps = in_maps
    return prep


# ================================================================= builder
def build(static):
    import concourse.bass as bass
    from concourse import bacc, tile
    import concourse.mybir as mybir

    cfg = static["cfg"]
    F = cfg["F"]
    ST_D, K, W_SUB = cfg["ST_D"], cfg["K"], cfg["W_SUB"]
    CW = F + W_SUB
    NST, NSUB = static["NST"], static["NSUB"]
    CH, TOT = static["CH"], static["TOT"]
    grans, ci_of = static["grans"], static["ci_of"]
    HID, EH, GL, GP = static["HID"], static["EH"], static["GL"], static["GP"]
    NCLS = cfg["NCLS"]
    NEG = cfg["NEG"]

    bdt = {"f32": mybir.dt.float32, "bf16": mybir.dt.bfloat16,
           "f8e4": mybir.dt.float8e4}
    gdt = bdt[cfg["GDT"]]
    mmdt = bdt[cfg["MMDT"]]
    f32 = mybir.dt.float32
    AF = mybir.ActivationFunctionType

    nc = bacc.Bacc(None, target_bir_lowering=False, debug=False)

    xs_d = nc.declare_dram_parameter("xs", [K, TOT * CW], gdt, isOutput=False)
    pmat_d = nc.declare_dram_parameter("pmat", [128, NST * GP], mmdt, isOutput=False)
    w1_d = nc.declare_dram_parameter("w1", [F, HID], mmdt, isOutput=False)
    w2r_d = nc.declare_dram_parameter("w2r", [128, 3 * HID], mmdt, isOutput=False)
    b1bc_d = nc.declare_dram_parameter("b1bc", [128, HID], mmdt, isOutput=False)
    b2bc_d = nc.declare_dram_parameter("b2bc", [GP, HID], mmdt, isOutput=False)
    w3bc_d = nc.declare_dram_parameter("w3bc", [GP, HID], mmdt, isOutput=False)
    b3bc_d = nc.declare_dram_parameter("b3bc", [GP, 1], f32, isOutput=False)
    emb_d = nc.declare_dram_parameter("embh", [NCLS, EH], mmdt, isOutput=False)
    clt_d = nc.declare_dram_parameter("clt", [NCLS, GP], mmdt, isOutput=False)
    out_d = nc.declare_dram_parameter("out", [GL, 1], f32, isOutput=True)

    iden_np = np.eye(32, dtype=_np_dt(cfg["MMDT"]))
    iden_d = nc.inline_tensor(iden_np, name="iden")

    with tile.TileContext(nc) as tc:
        with (
            tc.tile_pool(name="const", bufs=1) as constp,
            tc.tile_pool(name="xs", bufs=cfg["BUFS"]) as xsp,
            tc.tile_pool(name="work", bufs=3) as workp,
            tc.tile_pool(name="ps_agg", bufs=2, space="PSUM") as ps_agg,
            tc.tile_pool(name="ps_t", bufs=2, space="PSUM") as ps_t,
            tc.tile_pool(name="ps_h", bufs=2, space="PSUM") as ps_h,
            tc.tile_pool(name="ps_pool", bufs=1, space="PSUM") as ps_pool,
        ):
            # ---- data stream: all granules on the sync HWDGE ring
            xs_tiles = {}

            def issue_gran(gi):
                gm = grans[gi]
                ci0, nch = gm["ci0"], gm["nch"]
                xs = xsp.tile([K, nch, CW], gdt, tag="xs")
                nc.sync.dma_start(
                    out=xs[:, :, :],
                    in_=xs_d[:, ci0 * CW:(ci0 + nch) * CW])
                xs_tiles[gi] = xs

            issue_gran(0)
            issue_gran(1)

            # ---- persistent SBUF loads on the scalar ring (behind data)
            pmat_sb = constp.tile([128, NST * GP], mmdt)
            nc.scalar.dma_start(out=pmat_sb[:, :], in_=pmat_d[:, :])
            w1_sb = constp.tile([F, HID], mmdt)
            nc.scalar.dma_start(out=w1_sb[:, :], in_=w1_d[:, :])
            b1bc_sb = constp.tile([128, HID], mmdt)
            nc.scalar.dma_start(out=b1bc_sb[:, :], in_=b1bc_d[:, :])
            iden_sb = constp.tile([32, 32], mmdt)
            nc.scalar.dma_start(out=iden_sb[:, :], in_=iden_d[:, :])
            w2r_sb = constp.tile([128, 3 * HID], mmdt)
            nc.scalar.dma_start(out=w2r_sb[:, :], in_=w2r_d[:, :])
            b2bc_sb = constp.tile([GP, HID], mmdt)
            nc.scalar.dma_start(out=b2bc_sb[:, :], in_=b2bc_d[:, :])
            w3bc_sb = constp.tile([GP, HID], mmdt)
            nc.scalar.dma_start(out=w3bc_sb[:, :], in_=w3bc_d[:, :])
            b3bc_sb = constp.tile([GP, 1], f32)
            nc.scalar.dma_start(out=b3bc_sb[:, :], in_=b3bc_d[:, :])
            emb_sb = constp.tile([NCLS, EH], mmdt)
            nc.scalar.dma_start(out=emb_sb[:, :], in_=emb_d[:, :])
            clt_sb = constp.tile([NCLS, GP], mmdt)
            nc.scalar.dma_start(out=clt_sb[:, :], in_=clt_d[:, :])

            pooled_ps = ps_pool.tile([GP, HID], f32)

            def leaky(dst_pool, tag, src_ps, bias_sb, np_, wid):
                """dst = leaky_relu(src_ps + bias), cast to mmdt."""
                t_sb = workp.tile([np_, wid], f32, tag=f"t_{tag}")
                nc.vector.scalar_tensor_tensor(
                    t_sb[:, :], in0=src_ps[:, :], scalar=1.0,
                    in1=bias_sb[:, :], op0=mybir.AluOpType.mult,
                    op1=mybir.AluOpType.add)
                o_sb = workp.tile([np_, wid], mmdt, tag=f"o_{tag}")
                if cfg["PRELU"]:
                    nc.scalar.activation(o_sb[:, :], t_sb[:, :], AF.Prelu,
                                         alpha=NEG)
                else:
                    hr_sb = workp.tile([np_, wid], f32, tag=f"r_{tag}")
                    nc.scalar.activation(hr_sb[:, :], t_sb[:, :], AF.Relu,
                                         scale=1.0 - NEG)
                    nc.vector.scalar_tensor_tensor(
                        o_sb[:, :], in0=t_sb[:, :], scalar=NEG,
                        in1=hr_sb[:, :], op0=mybir.AluOpType.mult,
                        op1=mybir.AluOpType.add)
                return o_sb

            # per-supertile epilogue, software-pipelined one supertile
            # behind the S-matmuls so the PSUM->SBUF copy never stalls PE
            def emit_tail(st, agg):
                aggT_sb = workp.tile([128, 128], mmdt, tag="aggT_sb")
                nc.vector.tensor_copy(out=aggT_sb[:, :], in_=agg[:, :])
                h_ps = ps_h.tile([128, HID], f32, tag="h")
                nc.tensor.matmul(h_ps[:, :], lhsT=aggT_sb[:, :],
                                 rhs=w1_sb[:, :], start=True, stop=True)
                h_sb = leaky("h", "h", h_ps, b1bc_sb, 128, HID)
                # mean-pool accumulate (pmat carries 1/count)
                nc.tensor.matmul(
                    pooled_ps[:, :],
                    lhsT=pmat_sb[:, st * GP:(st + 1) * GP],
                    rhs=h_sb[:, :],
                    start=(st == 0), stop=(st == NST - 1),
                    skip_group_check=True)

            pending = []
            # ---------------- main loop over granules
            for gi, gm in enumerate(grans):
                st0, nst = gm["st0"], gm["nst"]
                ci0 = gm["ci0"]
                if gi not in xs_tiles:
                    issue_gran(gi)
                xs = xs_tiles.pop(gi)

                for st in range(st0, st0 + nst):
                    # S-matmuls: agg^T[feat, dst] accumulated per W_SUB-wide
                    # dst subrange (PSUM column slice; per-element
                    # has_written makes column-sliced groups independent)
                    agg = ps_agg.tile([128, ST_D], f32, tag="agg")
                    for r in range(NSUB):
                        nck = int(CH[st, r])
                        cib = int(ci_of[st, r]) - ci0
                        for k in range(nck):
                            ci = cib + k
                            nc.tensor.matmul(
                                agg[:, r * W_SUB:(r + 1) * W_SUB],
                                lhsT=xs[:, ci, 0:F],
                                rhs=xs[:, ci, F:CW],
                                start=(k == 0), stop=(k == nck - 1),
                                skip_group_check=True)
                    pending.append((st, agg))
                    if len(pending) > 1:
                        emit_tail(*pending.pop(0))
            for p in pending:
                emit_tail(*p)

            # ---------------- tail: core-local MLP on GL graphs
            pm_sb = workp.tile([GP, HID], mmdt, tag="pm")
            nc.vector.tensor_copy(out=pm_sb[:, :], in_=pooled_ps[:, :])

            # z^T rows [128, GP]: two transposed pooled halves + embeddings
            zt = []
            for jj in range(HID // 128):
                tp = ps_t.tile([128, GP], mmdt, tag="tp")
                nc.tensor.transpose(tp[:, :], pm_sb[:, jj * 128:(jj + 1) * 128],
                                    iden_sb[:, :])
                t_sb = workp.tile([128, GP], mmdt, tag=f"zt{jj}")
                nc.scalar.copy(out=t_sb[:, :], in_=tp[:, :])
                zt.append(t_sb)
            ce_ps = ps_t.tile([EH, GP], f32, tag="tp")
            nc.tensor.matmul(ce_ps[:, :], lhsT=emb_sb[:, :], rhs=clt_sb[:, :],
                             start=True, stop=True)
            ce_sb = workp.tile([EH, GP], mmdt, tag="ce_sb")
            nc.scalar.copy(out=ce_sb[:, :], in_=ce_ps[:, :])
            zt.append(ce_sb)

            # z2[g, hid] = sum_k zt[k].T @ W2rows[k]  (zt stationary)
            z2_ps = ps_h.tile([GP, HID], f32, tag="h")
            for kk in range(3):
                nc.tensor.matmul(
                    z2_ps[:, :], lhsT=zt[kk][:, :],
                    rhs=w2r_sb[:, kk * HID:(kk + 1) * HID],
                    start=(kk == 0), stop=(kk == 2))
            z2_sb = leaky("z2", "z2", z2_ps, b2bc_sb, GP, HID)

            # out[g] = sum_hid z2[g, hid] * W3[hid]  + b3 (DVE row-reduce)
            w3s_sb = workp.tile([GP, HID], mmdt, tag="w3s")
            res_sb = workp.tile([GP, 1], f32, tag="res")
            nc.vector.scalar_tensor_tensor(
                w3s_sb[:, :], in0=z2_sb[:, :], scalar=1.0,
                in1=w3bc_sb[:, :], op0=mybir.AluOpType.mult,
                op1=mybir.AluOpType.mult,
                accum_out=res_sb[:, :])
            o_sb = workp.tile([GP, 1], f32, tag="osb")
            nc.scalar.add(out=o_sb[:, :], in_=res_sb[:, :],
                          add=b3bc_sb[:, :])
            nc.sync.dma_start(out=out_d[:, :], in_=o_sb[0:GL, :])

    return nc


# ================================================================= runner
def _run(inputs, cfg=None, trace=False):
    from concourse.bass_utils import run_bass_kernel_spmd
    cfg = dict(CFG if cfg is None else cfg)
    prep = host_prep(inputs, cfg)
    nc = build(prep.static)
    nc.finalize()
    res = run_bass_kernel_spmd(
        nc, prep.in_maps, core_ids=list(range(cfg["NCORES"])), trace=trace)
    G = cfg["G"]
    out = np.zeros((G, 1), np.float32)
    for c, r in enumerate(res.results):
        oc = np.asarray(r["out"], np.float32).reshape(-1)
        for j, g in enumerate(prep.static["asg"][c]):
            out[g, 0] = oc[j]
    return out, res


def kernel(**inputs):
    out, _ = _run(inputs)
    return out


# revision 28
# speedup vs baseline: 1.1202x; 1.1202x over previous
"""GCN discriminator kernel for Trainium2 (8 NeuronCores, SPMD).

Math (matching the reference):
  deg[n]  = sum_{e: dst=n} w_e + 1
  dinv    = 1/sqrt(deg)
  norm_e  = dinv[src]*w_e*dinv[dst];  self-loop n: dinv[n]^2
  agg     = sum over incoming edges of norm_e * x[src]         [N, 128]
  h       = leaky_relu(agg @ W1 + b1)                          [N, 256]
  pooled  = segment_mean(h, batch)                             [64, 256]
  z       = leaky_relu(concat(pooled, emb[cls]) @ W2 + b2)
  out     = z @ W3 + b3                                        [64, 1]

Sharding: graph-aligned.  batch is sorted, so graphs occupy contiguous node
ranges; each core owns 8 whole graphs (LPT-balanced).  Pooling is core-local;
each core runs the tiny MLP on its own 8 graphs and the host concatenates.

Aggregation strategy: the host expands x rows into edge-slot order so the
device streams them with plain dense HWDGE DMAs — no SWDGE descriptor
emission (the original gather path was GPSIMD-emission-bound).  Self-loops
are folded in as ordinary edges (src=dst, w=dinv^2).  Edge slots are
bucketed by (dst supertile, W_SUB-wide dst subrange) so the one-hot S
matrices are [128 slots, W_SUB dsts]; x-rows and S interleave per chunk in
one contiguous stream (single DMA per granule, continuous SDMA flow).  Each
(supertile, subrange) accumulates in a PSUM column slice; per-element
has_written semantics make interleaved column-range groups safe.  Both
x-rows and S stream as fp8e4 (numerics verified offline: rel err ~1.9e-3 vs
1.77e-3 for bf16 — the bf16 downstream dominates).  The per-supertile
epilogue (PSUM evac, W1, bias, leaky, pool) is software-pipelined one
supertile behind the S-matmuls so its latency never stalls the PE.
"""

import numpy as np
import ml_dtypes

# ----------------------------------------------------------------- config
CFG = dict(
    N=50000, F=128, HID=256, G=64, NCLS=10,
    NCORES=8,
    ST_D=128,             # dsts per supertile (= PSUM agg width)
    W_SUB=32,             # dst subrange width (= S matmul free dim)
    K=128,                # edge slots per chunk (matmul contraction)
    GRAN_ST=12,           # supertiles per DMA granule (steady state)
    BUFS=3,               # stream tile-pool depth
    NEG=0.2,
    GDT="f8e4",           # x-row + S dtype: "f32" | "bf16" | "f8e4"
    MMDT="bf16",          # downstream matmul dtype
    PRELU=1,              # leaky relu via ACT Prelu (0: Relu + DVE blend)
)


def _np_dt(s):
    return {"f32": np.float32, "bf16": ml_dtypes.bfloat16,
            "f8e4": ml_dtypes.float8_e4m3}[s]


# ================================================================= host prep
class Prep:
    pass


def host_prep(inputs, cfg):
    """Integer/layout preprocessing + normalization weights.

    Returns per-core in_maps plus the static (core-independent) program
    structure.
    """
    N, F, G = cfg["N"], cfg["F"], cfg["G"]
    NC = cfg["NCORES"]
    ST_D, K, W_SUB = cfg["ST_D"], cfg["K"], cfg["W_SUB"]
    NSUB = ST_D // W_SUB
    CW = F + W_SUB                                 # stream cols per chunk
    GL = G // NC                                   # graphs per core
    GP = 32                                        # padded graph slots

    x = np.asarray(inputs["x"], np.float32)
    ei = np.asarray(inputs["edge_index"], np.int64)
    ew = np.asarray(inputs["edge_weight"], np.float32)
    batch = np.asarray(inputs["batch"], np.int64)
    cls = np.asarray(inputs["class_labels"], np.int64)
    W1 = np.asarray(inputs["W1"], np.float32)
    b1 = np.asarray(inputs["b1"], np.float32)
    emb = np.asarray(inputs["emb"], np.float32)
    W2 = np.asarray(inputs["W2"], np.float32)
    b2 = np.asarray(inputs["b2"], np.float32)
    W3 = np.asarray(inputs["W3"], np.float32)
    b3 = np.asarray(inputs["b3"], np.float32)

    HID = W1.shape[1]
    EH = emb.shape[1]

    # --- normalization weights (scalar preprocessing, O(E)) -------------
    row = ei[0]
    col = ei[1]
    deg = np.zeros(N, np.float64)
    np.add.at(deg, col, ew.astype(np.float64))
    deg += 1.0
    dinv = 1.0 / np.sqrt(deg)
    # augmented edge list: real edges + self loops
    a_src = np.concatenate([row, np.arange(N)])
    a_dst = np.concatenate([col, np.arange(N)])
    a_w = np.concatenate([
        (dinv[row] * ew.astype(np.float64) * dinv[col]).astype(np.float32),
        (dinv * dinv).astype(np.float32)])

    # --- balanced graph->core assignment (LPT bin packing, 8 per core) ----
    gcnt = np.bincount(batch, minlength=G)           # nodes per graph
    gcum = np.concatenate([[0], np.cumsum(gcnt)])    # graph start offsets
    load = np.zeros(NC, np.int64)
    nfill = np.zeros(NC, np.int64)
    asg = [[] for _ in range(NC)]                    # graphs per core
    for g in np.argsort(-gcnt, kind="stable"):
        c = min((c for c in range(NC) if nfill[c] < GL),
                key=lambda c: load[c])
        asg[c].append(int(g))
        load[c] += gcnt[g]
        nfill[c] += 1
    # in-degree incl. self loop, for slot-packing order
    indeg = np.bincount(col, minlength=N) + 1
    Dmax = int(load.max())
    NST = -(-Dmax // ST_D)      # supertiles per core
    NBK = NST * NSUB            # W_SUB-dst buckets per core

    def pack_core(d, caps):
        """Assign dsts (slot-counts d) to 32-dst buckets under caps.

        Greedy largest-that-fits per bucket; the same static cap profile
        across cores keeps the max-over-cores chunk counts tight."""
        order = np.argsort(-d, kind="stable")
        remaining = list(d[order])
        ridx = list(order)
        assign = np.empty(len(d), np.int64)
        achieved = np.zeros(len(caps), np.int64)
        for b, cap in enumerate(caps):
            if not remaining:
                break
            cnt, ssum = 0, 0
            i = 0
            while cnt < W_SUB and i < len(remaining):
                need_rest = W_SUB - cnt - 1
                tail = sum(remaining[-need_rest:]) if need_rest > 0 else 0
                if ssum + remaining[i] + tail <= cap:
                    ssum += remaining.pop(i)
                    assign[ridx.pop(i)] = b
                    cnt += 1
                else:
                    i += 1
            while cnt < W_SUB and remaining:
                ssum += remaining.pop()
                assign[ridx.pop()] = b
                cnt += 1
            achieved[b] = ssum
        return assign, achieved

    core_nls = [np.concatenate(
        [np.arange(gcum[g], gcum[g + 1]) for g in asg[c]]) for c in range(NC)]
    # static capacity profile: n5 five-chunk buckets then four-chunk ones;
    # small grid search, chunk totals decide
    base = max(0, -(-int(max(indeg[nl].sum() for nl in core_nls)
                         - NBK * 4 * K) // K))
    best = None
    for n5 in {base, base + 8, base + 16, base + 24}:
        n5 = min(n5, NBK)
        caps = np.array([5 * K] * n5 + [4 * K] * (NBK - n5))
        assigns = []
        CHp = np.zeros((NC, NBK), np.int64)
        for c in range(NC):
            a, ach = pack_core(indeg[core_nls[c]], caps)
            assigns.append(a)
            CHp[c] = -(-ach // K)
        tot = int(np.maximum(CHp.max(axis=0), 1).sum())
        if best is None or tot < best[0]:
            best = (tot, assigns, caps)
    # place small buckets first so leading granules (and the matmul
    # ramp-up) cover less data
    _, assigns, caps = best
    rank = np.empty(NBK, np.int64)
    rank[np.argsort(caps, kind="stable")] = np.arange(NBK)

    # local slot maps with holes: dst i sits at exactly
    # bucket*W_SUB + offset, partially-filled buckets leave ghost slots
    locmaps = []                                     # per-core loc -> node
    core_of_node = np.empty(N, np.int64)
    loc_of_node = np.empty(N, np.int64)
    for c in range(NC):
        nl = core_nls[c]
        fb = rank[assigns[c]]
        order = np.argsort(fb, kind="stable")
        fbo = fb[order]
        off = np.arange(len(fbo)) - np.searchsorted(fbo, fbo)
        locs = fbo * W_SUB + off
        locmap = np.full(NST * ST_D, -1, np.int64)
        locmap[locs] = nl[order]
        locmaps.append(locmap)
        core_of_node[nl[order]] = c
        loc_of_node[nl[order]] = locs

    # --- bucket edges into (core, st, sub) -------------------------------
    core_of = core_of_node[a_dst]
    dst_loc = loc_of_node[a_dst]
    st_of = dst_loc // ST_D
    sub_of = (dst_loc % ST_D) // W_SUB
    jj_of = dst_loc % W_SUB

    nbuckets = NC * NST * NSUB
    key = (core_of * NST + st_of) * NSUB + sub_of
    cnt = np.bincount(key, minlength=nbuckets).reshape(NC, NST, NSUB)
    starts = np.zeros(nbuckets + 1, np.int64)
    np.cumsum(cnt.reshape(-1), out=starts[1:])
    order = np.argsort(key, kind="stable")           # edges bucket-sorted
    e_src = a_src[order]
    e_jj = jj_of[order]
    e_w = a_w[order]

    # static chunk counts per (st, sub): max over cores, >= 1
    CH = np.maximum(-(-cnt // K), 1).max(axis=0)     # [NST, NSUB]
    TOT = int(CH.sum())                              # chunks per core

    # chunk index map in static issue order
    ci_of = np.zeros((NST, NSUB), np.int64)
    ci = 0
    for st in range(NST):
        for r in range(NSUB):
            ci_of[st, r] = ci
            ci += int(CH[st, r])

    # granules: small leading granules so the first matmuls start early,
    # then GRAN_ST supertiles per DMA for packet efficiency
    sizes = []
    st = 0
    for want in [1, 1, 1, 2, 2, 4, 4, 8]:
        if st < NST:
            n = min(want, NST - st)
            sizes.append(n)
            st += n
    while st < NST:
        n = min(cfg["GRAN_ST"], NST - st)
        sizes.append(n)
        st += n
    grans = []
    st = 0
    for n in sizes:
        ci0 = int(ci_of[st, 0])
        nch = int(CH[st:st + n].sum())
        grans.append(dict(st0=st, nst=n, ci0=ci0, nch=nch))
        st += n

    static = dict(cfg=cfg, NST=NST, NSUB=NSUB, CH=CH, TOT=TOT,
                  grans=grans, ci_of=ci_of,
                  HID=HID, EH=EH, GL=GL, GP=GP, asg=asg)

    # --- per-core tensors ------------------------------------------------
    gdt = _np_dt(cfg["GDT"])
    mmdt = _np_dt(cfg["MMDT"])

    counts = np.maximum(gcnt, 1).astype(np.float64)

    # W2 row blocks: [128, 3*256] with block k = W2[k*128:(k+1)*128, :]
    w2r = np.ascontiguousarray(
        W2.reshape(3, 128, HID).transpose(1, 0, 2).reshape(128, 3 * HID))

    x_q = np.ascontiguousarray(x).astype(gdt)

    in_maps = []
    for c in range(NC):
        # slot tables [TOT, K]: src index and (jj, w) per slot
        slots_src = np.zeros((TOT, K), np.int64)
        slots_jj = np.zeros((TOT, K), np.int64)
        slots_w = np.zeros((TOT, K), np.float32)
        for st in range(NST):
            for r in range(NSUB):
                b = (c * NST + st) * NSUB + r
                s0, s1 = starts[b], starts[b + 1]
                nck = int(CH[st, r])
                ci0 = int(ci_of[st, r])
                n = s1 - s0
                assert n <= nck * K
                flat = np.arange(n)
                cix = ci0 + flat // K
                pix = flat % K
                slots_src[cix, pix] = e_src[s0:s1]
                slots_jj[cix, pix] = e_jj[s0:s1]
                slots_w[cix, pix] = e_w[s0:s1]
        # interleaved stream [128, TOT*CW]: per chunk F x-row cols + W_SUB
        # one-hot S cols
        xs = np.zeros((TOT, K, CW), np.float32)
        xs[:, :, :F] = x_q[slots_src].astype(np.float32)
        tix = np.repeat(np.arange(TOT), K)
        pix = np.tile(np.arange(K), TOT)
        xs[tix, pix, F + slots_jj.reshape(-1)] = slots_w.reshape(-1)
        xs = np.ascontiguousarray(
            xs.transpose(1, 0, 2).reshape(K, TOT * CW)).astype(gdt)

        locmap = locmaps[c]
        gslot = {g: j for j, g in enumerate(asg[c])}

        # pooling matrix [128, NST*GP], scaled by 1/count (mean pooling);
        # ghost slots (locmap < 0) stay zero
        pmat = np.zeros((128, NST * GP), np.float64)
        for loc in np.nonzero(locmap >= 0)[0]:
            g = batch[locmap[loc]]
            pmat[loc % ST_D, (loc // ST_D) * GP + gslot[g]] = 1.0 / counts[g]

        # class embeddings for this core's graphs: clt [NCLS, GP]
        clt = np.zeros((cfg["NCLS"], GP), mmdt)
        clt[cls[asg[c]], np.arange(GL)] = 1.0

        m = dict(
            xs=xs,
            pmat=pmat.astype(mmdt),
            w1=W1.astype(mmdt),
            w2r=w2r.astype(mmdt),
            b1bc=np.tile(b1.reshape(1, HID), (128, 1)).astype(mmdt),
            b2bc=np.tile(b2.reshape(1, HID), (GP, 1)).astype(mmdt),
            w3bc=np.tile(W3.reshape(1, HID), (GP, 1)).astype(mmdt),
            b3bc=np.full((GP, 1), b3[0], np.float32),
            embh=emb.astype(mmdt),
            clt=clt,
        )
        in_maps.append(m)

    prep = Prep()
    prep.static = static
    prep.in_maps = in_maps
    return prep


# ================================================================= builder
def build(static):
    import concourse.bass as bass
    from concourse import bacc, tile
    import concourse.mybir as mybir

    cfg = static["cfg"]
    F = cfg["F"]
    ST_D, K, W_SUB = cfg["ST_D"], cfg["K"], cfg["W_SUB"]
    CW = F + W_SUB
    NST, NSUB = static["NST"], static["NSUB"]
    CH, TOT = static["CH"], static["TOT"]
    grans, ci_of = static["grans"], static["ci_of"]
    HID, EH, GL, GP = static["HID"], static["EH"], static["GL"], static["GP"]
    NCLS = cfg["NCLS"]
    NEG = cfg["NEG"]

    bdt = {"f32": mybir.dt.float32, "bf16": mybir.dt.bfloat16,
           "f8e4": mybir.dt.float8e4}
    gdt = bdt[cfg["GDT"]]
    mmdt = bdt[cfg["MMDT"]]
    f32 = mybir.dt.float32
    AF = mybir.ActivationFunctionType

    nc = bacc.Bacc(None, target_bir_lowering=False, debug=False)

    xs_d = nc.declare_dram_parameter("xs", [K, TOT * CW], gdt, isOutput=False)
    pmat_d = nc.declare_dram_parameter("pmat", [128, NST * GP], mmdt, isOutput=False)
    w1_d = nc.declare_dram_parameter("w1", [F, HID], mmdt, isOutput=False)
    w2r_d = nc.declare_dram_parameter("w2r", [128, 3 * HID], mmdt, isOutput=False)
    b1bc_d = nc.declare_dram_parameter("b1bc", [128, HID], mmdt, isOutput=False)
    b2bc_d = nc.declare_dram_parameter("b2bc", [GP, HID], mmdt, isOutput=False)
    w3bc_d = nc.declare_dram_parameter("w3bc", [GP, HID], mmdt, isOutput=False)
    b3bc_d = nc.declare_dram_parameter("b3bc", [GP, 1], f32, isOutput=False)
    emb_d = nc.declare_dram_parameter("embh", [NCLS, EH], mmdt, isOutput=False)
    clt_d = nc.declare_dram_parameter("clt", [NCLS, GP], mmdt, isOutput=False)
    out_d = nc.declare_dram_parameter("out", [GL, 1], f32, isOutput=True)

    iden_np = np.eye(32, dtype=_np_dt(cfg["MMDT"]))
    iden_d = nc.inline_tensor(iden_np, name="iden")

    with tile.TileContext(nc) as tc:
        with (
            tc.tile_pool(name="const", bufs=1) as constp,
            tc.tile_pool(name="xs", bufs=cfg["BUFS"]) as xsp,
            tc.tile_pool(name="work", bufs=3) as workp,
            tc.tile_pool(name="ps_agg", bufs=2, space="PSUM") as ps_agg,
            tc.tile_pool(name="ps_t", bufs=2, space="PSUM") as ps_t,
            tc.tile_pool(name="ps_h", bufs=2, space="PSUM") as ps_h,
            tc.tile_pool(name="ps_pool", bufs=1, space="PSUM") as ps_pool,
        ):
            # ---- data stream: all granules on the sync HWDGE ring
            xs_tiles = {}

            def issue_gran(gi):
                gm = grans[gi]
                ci0, nch = gm["ci0"], gm["nch"]
                xs = xsp.tile([K, nch, CW], gdt, tag="xs")
                nc.sync.dma_start(
                    out=xs[:, :, :],
                    in_=xs_d[:, ci0 * CW:(ci0 + nch) * CW])
                xs_tiles[gi] = xs

            issue_gran(0)
            issue_gran(1)

            # ---- persistent SBUF loads on the scalar ring (behind data)
            pmat_sb = constp.tile([128, NST * GP], mmdt)
            nc.scalar.dma_start(out=pmat_sb[:, :], in_=pmat_d[:, :])
            w1_sb = constp.tile([F, HID], mmdt)
            nc.scalar.dma_start(out=w1_sb[:, :], in_=w1_d[:, :])
            b1bc_sb = constp.tile([128, HID], mmdt)
            nc.scalar.dma_start(out=b1bc_sb[:, :], in_=b1bc_d[:, :])
            iden_sb = constp.tile([32, 32], mmdt)
            nc.scalar.dma_start(out=iden_sb[:, :], in_=iden_d[:, :])
            w2r_sb = constp.tile([128, 3 * HID], mmdt)
            nc.scalar.dma_start(out=w2r_sb[:, :], in_=w2r_d[:, :])
            b2bc_sb = constp.tile([GP, HID], mmdt)
            nc.scalar.dma_start(out=b2bc_sb[:, :], in_=b2bc_d[:, :])
            w3bc_sb = constp.tile([GP, HID], mmdt)
            nc.scalar.dma_start(out=w3bc_sb[:, :], in_=w3bc_d[:, :])
            b3bc_sb = constp.tile([GP, 1], f32)
            nc.scalar.dma_start(out=b3bc_sb[:, :], in_=b3bc_d[:, :])
            emb_sb = constp.tile([NCLS, EH], mmdt)
            nc.scalar.dma_start(out=emb_sb[:, :], in_=emb_d[:, :])
            clt_sb = constp.tile([NCLS, GP], mmdt)
            nc.scalar.dma_start(out=clt_sb[:, :], in_=clt_d[:, :])

            pooled_ps = ps_pool.tile([GP, HID], f32)

            def leaky(dst_pool, tag, src_ps, bias_sb, np_, wid):
                """dst = leaky_relu(src_ps + bias), cast to mmdt."""
                t_sb = workp.tile([np_, wid], f32, tag=f"t_{tag}")
                nc.vector.scalar_tensor_tensor(
                    t_sb[:, :], in0=src_ps[:, :], scalar=1.0,
                    in1=bias_sb[:, :], op0=mybir.AluOpType.mult,
                    op1=mybir.AluOpType.add)
                o_sb = workp.tile([np_, wid], mmdt, tag=f"o_{tag}")
                if cfg["PRELU"]:
                    nc.scalar.activation(o_sb[:, :], t_sb[:, :], AF.Prelu,
                                         alpha=NEG)
                else:
                    hr_sb = workp.tile([np_, wid], f32, tag=f"r_{tag}")
                    nc.scalar.activation(hr_sb[:, :], t_sb[:, :], AF.Relu,
                                         scale=1.0 - NEG)
                    nc.vector.scalar_tensor_tensor(
                        o_sb[:, :], in0=t_sb[:, :], scalar=NEG,
                        in1=hr_sb[:, :], op0=mybir.AluOpType.mult,
                        op1=mybir.AluOpType.add)
                return o_sb

            # per-supertile epilogue, software-pipelined one supertile
            # behind the S-matmuls so the PSUM->SBUF copy never stalls PE
            def emit_tail(st, agg):
                aggT_sb = workp.tile([128, 128], mmdt, tag="aggT_sb")
                nc.vector.tensor_copy(out=aggT_sb[:, :], in_=agg[:, :])
                h_ps = ps_h.tile([128, HID], f32, tag="h")
                nc.tensor.matmul(h_ps[:, :], lhsT=aggT_sb[:, :],
                                 rhs=w1_sb[:, :], start=True, stop=True)
                h_sb = leaky("h", "h", h_ps, b1bc_sb, 128, HID)
                # mean-pool accumulate (pmat carries 1/count)
                nc.tensor.matmul(
                    pooled_ps[:, :],
                    lhsT=pmat_sb[:, st * GP:(st + 1) * GP],
                    rhs=h_sb[:, :],
                    start=(st == 0), stop=(st == NST - 1),
                    skip_group_check=True)

            pending = []
            # ---------------- main loop over granules
            for gi, gm in enumerate(grans):
                st0, nst = gm["st0"], gm["nst"]
                ci0 = gm["ci0"]
                if gi not in xs_tiles:
                    issue_gran(gi)
                xs = xs_tiles.pop(gi)

                for st in range(st0, st0 + nst):
                    # S-matmuls: agg^T[feat, dst] accumulated per W_SUB-wide
                    # dst subrange (PSUM column slice; per-element
                    # has_written makes column-sliced groups independent)
                    agg = ps_agg.tile([128, ST_D], f32, tag="agg")
                    for r in range(NSUB):
                        nck = int(CH[st, r])
                        cib = int(ci_of[st, r]) - ci0
                        for k in range(nck):
                            ci = cib + k
                            nc.tensor.matmul(
                                agg[:, r * W_SUB:(r + 1) * W_SUB],
                                lhsT=xs[:, ci, 0:F],
                                rhs=xs[:, ci, F:CW],
                                start=(k == 0), stop=(k == nck - 1),
                                skip_group_check=True)
                    pending.append((st, agg))
                    if len(pending) > 1:
                        emit_tail(*pending.pop(0))
            for p in pending:
                emit_tail(*p)

            # ---------------- tail: core-local MLP on GL graphs
            pm_sb = workp.tile([GP, HID], mmdt, tag="pm")
            nc.vector.tensor_copy(out=pm_sb[:, :], in_=pooled_ps[:, :])

            # z^T rows [128, GP]: two transposed pooled halves + embeddings
            zt = []
            for jj in range(HID // 128):
                tp = ps_t.tile([128, GP], mmdt, tag="tp")
                nc.tensor.transpose(tp[:, :], pm_sb[:, jj * 128:(jj + 1) * 128],
                                    iden_sb[:, :])
                t_sb = workp.tile([128, GP], mmdt, tag=f"zt{jj}")
                nc.scalar.copy(out=t_sb[:, :], in_=tp[:, :])
                zt.append(t_sb)
            ce_ps = ps_t.tile([EH, GP], f32, tag="tp")
            nc.tensor.matmul(ce_ps[:, :], lhsT=emb_sb[:, :], rhs=clt_sb[:, :],
                             start=True, stop=True)
            ce_sb = workp.tile([EH, GP], mmdt, tag="ce_sb")
            nc.scalar.copy(out=ce_sb[:, :], in_=ce_ps[:, :])
            zt.append(ce_sb)

            # z2[g, hid] = sum_k zt[k].T @ W2rows[k]  (zt stationary)
            z2_ps = ps_h.tile([GP, HID], f32, tag="h")
            for kk in range(3):
                nc.tensor.matmul(
                    z2_ps[:, :], lhsT=zt[kk][:, :],
                    rhs=w2r_sb[:, kk * HID:(kk + 1) * HID],
                    start=(kk == 0), stop=(kk == 2))
            z2_sb = leaky("z2", "z2", z2_ps, b2bc_sb, GP, HID)

            # out[g] = sum_hid z2[g, hid] * W3[hid]  + b3 (DVE row-reduce)
            w3s_sb = workp.tile([GP, HID], mmdt, tag="w3s")
            res_sb = workp.tile([GP, 1], f32, tag="res")
            nc.vector.scalar_tensor_tensor(
                w3s_sb[:, :], in0=z2_sb[:, :], scalar=1.0,
                in1=w3bc_sb[:, :], op0=mybir.AluOpType.mult,
                op1=mybir.AluOpType.mult,
                accum_out=res_sb[:, :])
            o_sb = workp.tile([GP, 1], f32, tag="osb")
            nc.scalar.add(out=o_sb[:, :], in_=res_sb[:, :],
                          add=b3bc_sb[:, :])
            nc.sync.dma_start(out=out_d[:, :], in_=o_sb[0:GL, :])

    return nc


# ================================================================= runner
def _run(inputs, cfg=None, trace=False):
    from concourse.bass_utils import run_bass_kernel_spmd
    cfg = dict(CFG if cfg is None else cfg)
    prep = host_prep(inputs, cfg)
    nc = build(prep.static)
    nc.finalize()
    res = run_bass_kernel_spmd(
        nc, prep.in_maps, core_ids=list(range(cfg["NCORES"])), trace=trace)
    G = cfg["G"]
    out = np.zeros((G, 1), np.float32)
    for c, r in enumerate(res.results):
        oc = np.asarray(r["out"], np.float32).reshape(-1)
        for j, g in enumerate(prep.static["asg"][c]):
            out[g, 0] = oc[j]
    return out, res


def kernel(**inputs):
    out, _ = _run(inputs)
    return out


# revision 33
# speedup vs baseline: 1.1611x; 1.0365x over previous
"""GCN discriminator kernel for Trainium2 (8 NeuronCores, SPMD).

Math (matching the reference):
  deg[n]  = sum_{e: dst=n} w_e + 1
  dinv    = 1/sqrt(deg)
  norm_e  = dinv[src]*w_e*dinv[dst];  self-loop n: dinv[n]^2
  agg     = sum over incoming edges of norm_e * x[src]         [N, 128]
  h       = leaky_relu(agg @ W1 + b1)                          [N, 256]
  pooled  = segment_mean(h, batch)                             [64, 256]
  z       = leaky_relu(concat(pooled, emb[cls]) @ W2 + b2)
  out     = z @ W3 + b3                                        [64, 1]

Sharding: graph-aligned.  batch is sorted, so graphs occupy contiguous node
ranges; each core owns 8 whole graphs (LPT-balanced).  Pooling is core-local;
each core runs the tiny MLP on its own 8 graphs and the host concatenates.

Aggregation strategy: the host expands x rows into edge-slot order so the
device streams them with plain dense HWDGE DMAs — no SWDGE descriptor
emission (the original gather path was GPSIMD-emission-bound).  Self-loops
are folded in as ordinary edges (src=dst, w=dinv^2).  Edge slots are
bucketed by (dst supertile, W_SUB-wide dst subrange) so the one-hot S
matrices are [128 slots, W_SUB dsts]; x-rows and S interleave per chunk in
one contiguous stream (single DMA per granule, continuous SDMA flow).  Each
(supertile, subrange) accumulates in a PSUM column slice; per-element
has_written semantics make interleaved column-range groups safe.  Both
x-rows and S stream as fp8e4 (numerics verified offline: rel err ~1.9e-3 vs
1.77e-3 for bf16 — the bf16 downstream dominates).  The per-supertile
epilogue (PSUM evac, W1, bias, leaky, pool) is software-pipelined one
supertile behind the S-matmuls so its latency never stalls the PE.
"""

import numpy as np
import ml_dtypes

# ----------------------------------------------------------------- config
CFG = dict(
    N=50000, F=128, HID=256, G=64, NCLS=10,
    NCORES=8,
    ST_D=128,             # dsts per supertile (= PSUM agg width)
    W_SUB=32,             # dst subrange width (= S matmul free dim)
    K=128,                # edge slots per chunk (matmul contraction)
    GRAN_ST=12,           # supertiles per DMA granule (steady state)
    BUFS=3,               # stream tile-pool depth
    NEG=0.2,
    GDT="f8e4",           # x-row + S dtype: "f32" | "bf16" | "f8e4"
    MMDT="bf16",          # downstream matmul dtype
    PRELU=1,              # leaky relu via ACT Prelu (0: Relu + DVE blend)
)


def _np_dt(s):
    return {"f32": np.float32, "bf16": ml_dtypes.bfloat16,
            "f8e4": ml_dtypes.float8_e4m3}[s]


# ================================================================= host prep
class Prep:
    pass


def host_prep(inputs, cfg):
    """Integer/layout preprocessing + normalization weights.

    Returns per-core in_maps plus the static (core-independent) program
    structure.
    """
    N, F, G = cfg["N"], cfg["F"], cfg["G"]
    NC = cfg["NCORES"]
    ST_D, K, W_SUB = cfg["ST_D"], cfg["K"], cfg["W_SUB"]
    NSUB = ST_D // W_SUB
    CW = F + W_SUB                                 # stream cols per chunk
    GL = G // NC                                   # graphs per core
    GP = 32                                        # padded graph slots

    x = np.asarray(inputs["x"], np.float32)
    ei = np.asarray(inputs["edge_index"], np.int64)
    ew = np.asarray(inputs["edge_weight"], np.float32)
    batch = np.asarray(inputs["batch"], np.int64)
    cls = np.asarray(inputs["class_labels"], np.int64)
    W1 = np.asarray(inputs["W1"], np.float32)
    b1 = np.asarray(inputs["b1"], np.float32)
    emb = np.asarray(inputs["emb"], np.float32)
    W2 = np.asarray(inputs["W2"], np.float32)
    b2 = np.asarray(inputs["b2"], np.float32)
    W3 = np.asarray(inputs["W3"], np.float32)
    b3 = np.asarray(inputs["b3"], np.float32)

    HID = W1.shape[1]
    EH = emb.shape[1]

    # --- normalization weights (scalar preprocessing, O(E)) -------------
    row = ei[0]
    col = ei[1]
    deg = np.zeros(N, np.float64)
    np.add.at(deg, col, ew.astype(np.float64))
    deg += 1.0
    dinv = 1.0 / np.sqrt(deg)
    # augmented edge list: real edges + self loops
    a_src = np.concatenate([row, np.arange(N)])
    a_dst = np.concatenate([col, np.arange(N)])
    a_w = np.concatenate([
        (dinv[row] * ew.astype(np.float64) * dinv[col]).astype(np.float32),
        (dinv * dinv).astype(np.float32)])

    # --- balanced graph->core assignment (LPT bin packing, 8 per core) ----
    gcnt = np.bincount(batch, minlength=G)           # nodes per graph
    gcum = np.concatenate([[0], np.cumsum(gcnt)])    # graph start offsets
    load = np.zeros(NC, np.int64)
    nfill = np.zeros(NC, np.int64)
    asg = [[] for _ in range(NC)]                    # graphs per core
    for g in np.argsort(-gcnt, kind="stable"):
        c = min((c for c in range(NC) if nfill[c] < GL),
                key=lambda c: load[c])
        asg[c].append(int(g))
        load[c] += gcnt[g]
        nfill[c] += 1
    # in-degree incl. self loop, for slot-packing order
    indeg = np.bincount(col, minlength=N) + 1
    Dmax = int(load.max())
    NST = -(-Dmax // ST_D)      # supertiles per core
    NBK = NST * NSUB            # W_SUB-dst buckets per core

    def pack_core(d, caps):
        """Assign dsts (slot-counts d) to 32-dst buckets under caps.

        Greedy largest-that-fits per bucket; the same static cap profile
        across cores keeps the max-over-cores chunk counts tight."""
        order = np.argsort(-d, kind="stable")
        remaining = list(d[order])
        ridx = list(order)
        assign = np.empty(len(d), np.int64)
        achieved = np.zeros(len(caps), np.int64)
        for b, cap in enumerate(caps):
            if not remaining:
                break
            cnt, ssum = 0, 0
            i = 0
            while cnt < W_SUB and i < len(remaining):
                need_rest = W_SUB - cnt - 1
                tail = sum(remaining[-need_rest:]) if need_rest > 0 else 0
                if ssum + remaining[i] + tail <= cap:
                    ssum += remaining.pop(i)
                    assign[ridx.pop(i)] = b
                    cnt += 1
                else:
                    i += 1
            while cnt < W_SUB and remaining:
                ssum += remaining.pop()
                assign[ridx.pop()] = b
                cnt += 1
            achieved[b] = ssum
        return assign, achieved

    core_nls = [np.concatenate(
        [np.arange(gcum[g], gcum[g + 1]) for g in asg[c]]) for c in range(NC)]
    # static capacity profile: n5 five-chunk buckets then four-chunk ones;
    # small grid search, chunk totals decide
    base = max(0, -(-int(max(indeg[nl].sum() for nl in core_nls)
                         - NBK * 4 * K) // K))
    best = None
    for n5 in {base, base + 8, base + 16, base + 24}:
        n5 = min(n5, NBK)
        caps = np.array([5 * K] * n5 + [4 * K] * (NBK - n5))
        assigns = []
        CHp = np.zeros((NC, NBK), np.int64)
        for c in range(NC):
            a, ach = pack_core(indeg[core_nls[c]], caps)
            assigns.append(a)
            CHp[c] = -(-ach // K)
        tot = int(np.maximum(CHp.max(axis=0), 1).sum())
        if best is None or tot < best[0]:
            best = (tot, assigns, caps)
    # place small buckets first so leading granules (and the matmul
    # ramp-up) cover less data
    _, assigns, caps = best
    rank = np.empty(NBK, np.int64)
    rank[np.argsort(caps, kind="stable")] = np.arange(NBK)

    # local slot maps with holes: dst i sits at exactly
    # bucket*W_SUB + offset, partially-filled buckets leave ghost slots
    locmaps = []                                     # per-core loc -> node
    core_of_node = np.empty(N, np.int64)
    loc_of_node = np.empty(N, np.int64)
    for c in range(NC):
        nl = core_nls[c]
        fb = rank[assigns[c]]
        order = np.argsort(fb, kind="stable")
        fbo = fb[order]
        off = np.arange(len(fbo)) - np.searchsorted(fbo, fbo)
        locs = fbo * W_SUB + off
        locmap = np.full(NST * ST_D, -1, np.int64)
        locmap[locs] = nl[order]
        locmaps.append(locmap)
        core_of_node[nl[order]] = c
        loc_of_node[nl[order]] = locs

    # --- bucket edges into (core, st, sub) -------------------------------
    core_of = core_of_node[a_dst]
    dst_loc = loc_of_node[a_dst]
    st_of = dst_loc // ST_D
    sub_of = (dst_loc % ST_D) // W_SUB
    jj_of = dst_loc % W_SUB

    nbuckets = NC * NST * NSUB
    key = (core_of * NST + st_of) * NSUB + sub_of
    cnt = np.bincount(key, minlength=nbuckets).reshape(NC, NST, NSUB)
    starts = np.zeros(nbuckets + 1, np.int64)
    np.cumsum(cnt.reshape(-1), out=starts[1:])
    order = np.argsort(key, kind="stable")           # edges bucket-sorted
    e_src = a_src[order]
    e_jj = jj_of[order]
    e_w = a_w[order]

    # static chunk counts per (st, sub): max over cores, >= 1
    CH = np.maximum(-(-cnt // K), 1).max(axis=0)     # [NST, NSUB]
    TOT = int(CH.sum())                              # chunks per core

    # chunk index map in static issue order
    ci_of = np.zeros((NST, NSUB), np.int64)
    ci = 0
    for st in range(NST):
        for r in range(NSUB):
            ci_of[st, r] = ci
            ci += int(CH[st, r])

    # DMA groups: bucket-aligned chunk ranges, geometrically growing so the
    # first matmuls start on a single bucket's data, then GRAN_ST-supertile
    # sized groups for packet efficiency
    bucket_ch = [int(CH[st, r]) for st in range(NST) for r in range(NSUB)]
    big = cfg["GRAN_ST"] * TOT // NST
    targets = [4, 12, 16, 32, 32, 64, 64, 128]
    groups = []
    g_of = np.zeros(TOT, np.int64)
    ci = 0
    bi = 0
    ti = 0
    while bi < len(bucket_ch):
        tgt = targets[ti] if ti < len(targets) else big
        ti += 1
        nch = 0
        while bi < len(bucket_ch) and nch < tgt:
            nch += bucket_ch[bi]
            bi += 1
        groups.append(dict(ci0=ci, nch=nch))
        g_of[ci:ci + nch] = len(groups) - 1
        ci += nch

    static = dict(cfg=cfg, NST=NST, NSUB=NSUB, CH=CH, TOT=TOT,
                  groups=groups, g_of=g_of, ci_of=ci_of,
                  HID=HID, EH=EH, GL=GL, GP=GP, asg=asg)

    # --- per-core tensors ------------------------------------------------
    gdt = _np_dt(cfg["GDT"])
    mmdt = _np_dt(cfg["MMDT"])

    counts = np.maximum(gcnt, 1).astype(np.float64)

    # W2 row blocks: [128, 3*256] with block k = W2[k*128:(k+1)*128, :]
    w2r = np.ascontiguousarray(
        W2.reshape(3, 128, HID).transpose(1, 0, 2).reshape(128, 3 * HID))

    x_q = np.ascontiguousarray(x).astype(gdt)

    in_maps = []
    for c in range(NC):
        # slot tables [TOT, K]: src index and (jj, w) per slot
        slots_src = np.zeros((TOT, K), np.int64)
        slots_jj = np.zeros((TOT, K), np.int64)
        slots_w = np.zeros((TOT, K), np.float32)
        for st in range(NST):
            for r in range(NSUB):
                b = (c * NST + st) * NSUB + r
                s0, s1 = starts[b], starts[b + 1]
                nck = int(CH[st, r])
                ci0 = int(ci_of[st, r])
                n = s1 - s0
                assert n <= nck * K
                flat = np.arange(n)
                cix = ci0 + flat // K
                pix = flat % K
                slots_src[cix, pix] = e_src[s0:s1]
                slots_jj[cix, pix] = e_jj[s0:s1]
                slots_w[cix, pix] = e_w[s0:s1]
        # interleaved stream [128, TOT*CW]: per chunk F x-row cols + W_SUB
        # one-hot S cols
        xs = np.zeros((TOT, K, CW), np.float32)
        xs[:, :, :F] = x_q[slots_src].astype(np.float32)
        tix = np.repeat(np.arange(TOT), K)
        pix = np.tile(np.arange(K), TOT)
        xs[tix, pix, F + slots_jj.reshape(-1)] = slots_w.reshape(-1)
        xs = np.ascontiguousarray(
            xs.transpose(1, 0, 2).reshape(K, TOT * CW)).astype(gdt)

        locmap = locmaps[c]
        gslot = {g: j for j, g in enumerate(asg[c])}

        # pooling matrix [128, NST*GP], scaled by 1/count (mean pooling);
        # ghost slots (locmap < 0) stay zero
        pmat = np.zeros((128, NST * GP), np.float64)
        for loc in np.nonzero(locmap >= 0)[0]:
            g = batch[locmap[loc]]
            pmat[loc % ST_D, (loc // ST_D) * GP + gslot[g]] = 1.0 / counts[g]

        # class embeddings for this core's graphs: clt [NCLS, GP]
        clt = np.zeros((cfg["NCLS"], GP), mmdt)
        clt[cls[asg[c]], np.arange(GL)] = 1.0

        m = dict(
            xs=xs,
            pmat=pmat.astype(mmdt),
            w1=W1.astype(mmdt),
            w2r=w2r.astype(mmdt),
            b1bc=np.tile(b1.reshape(1, HID), (128, 1)).astype(mmdt),
            b2bc=np.tile(b2.reshape(1, HID), (GP, 1)).astype(mmdt),
            w3bc=np.tile(W3.reshape(1, HID), (GP, 1)).astype(mmdt),
            b3bc=np.full((GP, 1), b3[0], np.float32),
            embh=emb.astype(mmdt),
            clt=clt,
        )
        in_maps.append(m)

    prep = Prep()
    prep.static = static
    prep.in_maps = in_maps
    return prep


# ================================================================= builder
def build(static):
    import concourse.bass as bass
    from concourse import bacc, tile
    import concourse.mybir as mybir

    cfg = static["cfg"]
    F = cfg["F"]
    ST_D, K, W_SUB = cfg["ST_D"], cfg["K"], cfg["W_SUB"]
    CW = F + W_SUB
    NST, NSUB = static["NST"], static["NSUB"]
    CH, TOT = static["CH"], static["TOT"]
    groups, g_of = static["groups"], static["g_of"]
    ci_of = static["ci_of"]
    HID, EH, GL, GP = static["HID"], static["EH"], static["GL"], static["GP"]
    NCLS = cfg["NCLS"]
    NEG = cfg["NEG"]

    bdt = {"f32": mybir.dt.float32, "bf16": mybir.dt.bfloat16,
           "f8e4": mybir.dt.float8e4}
    gdt = bdt[cfg["GDT"]]
    mmdt = bdt[cfg["MMDT"]]
    f32 = mybir.dt.float32
    AF = mybir.ActivationFunctionType

    nc = bacc.Bacc(None, target_bir_lowering=False, debug=False)

    xs_d = nc.declare_dram_parameter("xs", [K, TOT * CW], gdt, isOutput=False)
    pmat_d = nc.declare_dram_parameter("pmat", [128, NST * GP], mmdt, isOutput=False)
    w1_d = nc.declare_dram_parameter("w1", [F, HID], mmdt, isOutput=False)
    w2r_d = nc.declare_dram_parameter("w2r", [128, 3 * HID], mmdt, isOutput=False)
    b1bc_d = nc.declare_dram_parameter("b1bc", [128, HID], mmdt, isOutput=False)
    b2bc_d = nc.declare_dram_parameter("b2bc", [GP, HID], mmdt, isOutput=False)
    w3bc_d = nc.declare_dram_parameter("w3bc", [GP, HID], mmdt, isOutput=False)
    b3bc_d = nc.declare_dram_parameter("b3bc", [GP, 1], f32, isOutput=False)
    emb_d = nc.declare_dram_parameter("embh", [NCLS, EH], mmdt, isOutput=False)
    clt_d = nc.declare_dram_parameter("clt", [NCLS, GP], mmdt, isOutput=False)
    out_d = nc.declare_dram_parameter("out", [GL, 1], f32, isOutput=True)

    iden_np = np.eye(32, dtype=_np_dt(cfg["MMDT"]))
    iden_d = nc.inline_tensor(iden_np, name="iden")

    with tile.TileContext(nc) as tc:
        with (
            tc.tile_pool(name="const", bufs=1) as constp,
            tc.tile_pool(name="xs", bufs=cfg["BUFS"]) as xsp,
            tc.tile_pool(name="work", bufs=3) as workp,
            tc.tile_pool(name="ps_agg", bufs=2, space="PSUM") as ps_agg,
            tc.tile_pool(name="ps_t", bufs=2, space="PSUM") as ps_t,
            tc.tile_pool(name="ps_h", bufs=2, space="PSUM") as ps_h,
            tc.tile_pool(name="ps_pool", bufs=1, space="PSUM") as ps_pool,
        ):
            # ---- data stream: all groups on the sync HWDGE ring
            xs_tiles = {}

            def issue_group(gi):
                gm = groups[gi]
                ci0, nch = gm["ci0"], gm["nch"]
                xs = xsp.tile([K, nch, CW], gdt, tag="xs")
                nc.sync.dma_start(
                    out=xs[:, :, :],
                    in_=xs_d[:, ci0 * CW:(ci0 + nch) * CW])
                xs_tiles[gi] = xs

            issue_group(0)
            issue_group(1)

            # ---- persistent SBUF loads on the scalar ring (behind data)
            pmat_sb = constp.tile([128, NST * GP], mmdt)
            nc.scalar.dma_start(out=pmat_sb[:, :], in_=pmat_d[:, :])
            w1_sb = constp.tile([F, HID], mmdt)
            nc.scalar.dma_start(out=w1_sb[:, :], in_=w1_d[:, :])
            b1bc_sb = constp.tile([128, HID], mmdt)
            nc.scalar.dma_start(out=b1bc_sb[:, :], in_=b1bc_d[:, :])
            iden_sb = constp.tile([32, 32], mmdt)
            nc.scalar.dma_start(out=iden_sb[:, :], in_=iden_d[:, :])
            w2r_sb = constp.tile([128, 3 * HID], mmdt)
            nc.scalar.dma_start(out=w2r_sb[:, :], in_=w2r_d[:, :])
            b2bc_sb = constp.tile([GP, HID], mmdt)
            nc.scalar.dma_start(out=b2bc_sb[:, :], in_=b2bc_d[:, :])
            w3bc_sb = constp.tile([GP, HID], mmdt)
            nc.scalar.dma_start(out=w3bc_sb[:, :], in_=w3bc_d[:, :])
            b3bc_sb = constp.tile([GP, 1], f32)
            nc.scalar.dma_start(out=b3bc_sb[:, :], in_=b3bc_d[:, :])
            emb_sb = constp.tile([NCLS, EH], mmdt)
            nc.scalar.dma_start(out=emb_sb[:, :], in_=emb_d[:, :])
            clt_sb = constp.tile([NCLS, GP], mmdt)
            nc.scalar.dma_start(out=clt_sb[:, :], in_=clt_d[:, :])

            pooled_ps = ps_pool.tile([GP, HID], f32)

            def leaky(dst_pool, tag, src_ps, bias_sb, np_, wid):
                """dst = leaky_relu(src_ps + bias), cast to mmdt."""
                t_sb = workp.tile([np_, wid], f32, tag=f"t_{tag}")
                nc.vector.scalar_tensor_tensor(
                    t_sb[:, :], in0=src_ps[:, :], scalar=1.0,
                    in1=bias_sb[:, :], op0=mybir.AluOpType.mult,
                    op1=mybir.AluOpType.add)
                o_sb = workp.tile([np_, wid], mmdt, tag=f"o_{tag}")
                if cfg["PRELU"]:
                    nc.scalar.activation(o_sb[:, :], t_sb[:, :], AF.Prelu,
                                         alpha=NEG)
                else:
                    hr_sb = workp.tile([np_, wid], f32, tag=f"r_{tag}")
                    nc.scalar.activation(hr_sb[:, :], t_sb[:, :], AF.Relu,
                                         scale=1.0 - NEG)
                    nc.vector.scalar_tensor_tensor(
                        o_sb[:, :], in0=t_sb[:, :], scalar=NEG,
                        in1=hr_sb[:, :], op0=mybir.AluOpType.mult,
                        op1=mybir.AluOpType.add)
                return o_sb

            # per-supertile epilogue, software-pipelined one supertile
            # behind the S-matmuls so the PSUM->SBUF copy never stalls PE
            def emit_tail(st, agg):
                aggT_sb = workp.tile([128, 128], mmdt, tag="aggT_sb")
                nc.vector.tensor_copy(out=aggT_sb[:, :], in_=agg[:, :])
                h_ps = ps_h.tile([128, HID], f32, tag="h")
                nc.tensor.matmul(h_ps[:, :], lhsT=aggT_sb[:, :],
                                 rhs=w1_sb[:, :], start=True, stop=True)
                h_sb = leaky("h", "h", h_ps, b1bc_sb, 128, HID)
                # mean-pool accumulate (pmat carries 1/count)
                nc.tensor.matmul(
                    pooled_ps[:, :],
                    lhsT=pmat_sb[:, st * GP:(st + 1) * GP],
                    rhs=h_sb[:, :],
                    start=(st == 0), stop=(st == NST - 1),
                    skip_group_check=True)

            pending = []
            # ---------------- main loop over supertiles
            for st in range(NST):
                # S-matmuls: agg^T[feat, dst] accumulated per W_SUB-wide
                # dst subrange (PSUM column slice; per-element has_written
                # makes column-sliced groups independent)
                agg = ps_agg.tile([128, ST_D], f32, tag="agg")
                for r in range(NSUB):
                    nck = int(CH[st, r])
                    cig = int(ci_of[st, r])
                    for k in range(nck):
                        ci = cig + k
                        gi = int(g_of[ci])
                        if gi not in xs_tiles:
                            issue_group(gi)
                        xs = xs_tiles[gi]
                        cil = ci - groups[gi]["ci0"]
                        nc.tensor.matmul(
                            agg[:, r * W_SUB:(r + 1) * W_SUB],
                            lhsT=xs[:, cil, 0:F],
                            rhs=xs[:, cil, F:CW],
                            start=(k == 0), stop=(k == nck - 1),
                            skip_group_check=True)
                pending.append((st, agg))
                if len(pending) > 1:
                    emit_tail(*pending.pop(0))
            for p in pending:
                emit_tail(*p)

            # ---------------- tail: core-local MLP on GL graphs
            pm_sb = workp.tile([GP, HID], mmdt, tag="pm")
            nc.vector.tensor_copy(out=pm_sb[:, :], in_=pooled_ps[:, :])

            # z^T rows [128, GP]: two transposed pooled halves + embeddings
            zt = []
            for jj in range(HID // 128):
                tp = ps_t.tile([128, GP], mmdt, tag="tp")
                nc.tensor.transpose(tp[:, :], pm_sb[:, jj * 128:(jj + 1) * 128],
                                    iden_sb[:, :])
                t_sb = workp.tile([128, GP], mmdt, tag=f"zt{jj}")
                nc.scalar.copy(out=t_sb[:, :], in_=tp[:, :])
                zt.append(t_sb)
            ce_ps = ps_t.tile([EH, GP], f32, tag="tp")
            nc.tensor.matmul(ce_ps[:, :], lhsT=emb_sb[:, :], rhs=clt_sb[:, :],
                             start=True, stop=True)
            ce_sb = workp.tile([EH, GP], mmdt, tag="ce_sb")
            nc.scalar.copy(out=ce_sb[:, :], in_=ce_ps[:, :])
            zt.append(ce_sb)

            # z2[g, hid] = sum_k zt[k].T @ W2rows[k]  (zt stationary)
            z2_ps = ps_h.tile([GP, HID], f32, tag="h")
            for kk in range(3):
                nc.tensor.matmul(
                    z2_ps[:, :], lhsT=zt[kk][:, :],
                    rhs=w2r_sb[:, kk * HID:(kk + 1) * HID],
                    start=(kk == 0), stop=(kk == 2))
            z2_sb = leaky("z2", "z2", z2_ps, b2bc_sb, GP, HID)

            # out[g] = sum_hid z2[g, hid] * W3[hid]  + b3 (DVE row-reduce)
            w3s_sb = workp.tile([GP, HID], mmdt, tag="w3s")
            res_sb = workp.tile([GP, 1], f32, tag="res")
            nc.vector.scalar_tensor_tensor(
                w3s_sb[:, :], in0=z2_sb[:, :], scalar=1.0,
                in1=w3bc_sb[:, :], op0=mybir.AluOpType.mult,
                op1=mybir.AluOpType.mult,
                accum_out=res_sb[:, :])
            o_sb = workp.tile([GP, 1], f32, tag="osb")
            nc.scalar.add(out=o_sb[:, :], in_=res_sb[:, :],
                          add=b3bc_sb[:, :])
            nc.sync.dma_start(out=out_d[:, :], in_=o_sb[0:GL, :])

    return nc


# ================================================================= runner
def _run(inputs, cfg=None, trace=False):
    from concourse.bass_utils import run_bass_kernel_spmd
    cfg = dict(CFG if cfg is None else cfg)
    prep = host_prep(inputs, cfg)
    nc = build(prep.static)
    nc.finalize()
    res = run_bass_kernel_spmd(
        nc, prep.in_maps, core_ids=list(range(cfg["NCORES"])), trace=trace)
    G = cfg["G"]
    out = np.zeros((G, 1), np.float32)
    for c, r in enumerate(res.results):
        oc = np.asarray(r["out"], np.float32).reshape(-1)
        for j, g in enumerate(prep.static["asg"][c]):
            out[g, 0] = oc[j]
    return out, res


def kernel(**inputs):
    out, _ = _run(inputs)
    return out
